# revision 3
# baseline (speedup 1.0000x reference)
"""Trainium2 Bass kernel for nn_DeformBottleneck (DCNv2 bottleneck block).

Contract: kernel(**inputs) takes the FULL unsharded inputs (batch 8) and
returns the FULL [8, 256, 80, 80] output. Internally: data-parallel over
batch across 8 NeuronCores (one sample per core), SPMD via
run_bass_kernel_spmd.

Algorithm (per core / sample), all shapes hardcoded:
  y = silu(BN1(cv1 @ x))                          # [128, 6400] channel-major
  raw = off_conv3x3(y); offsets/mask from raw
  DCNv2 via exact 3x3 "hat window" reformulation of bilinear sampling:
    out[o,p] = sum_n sum_{dy,dx} A[n,dy,dx,p] * (W_n y)[o, p+(dy,dx)]
  where A = mask * relu(1-|offy-dy|) * relu(1-|offx-dx|); zero-padding
  reproduces the reference's corner-validity masking exactly.
  Implemented as 81 accumulating matmuls whose rhs is y (shifted AP)
  pre-multiplied by the per-pixel weight row (replicated across the 128
  partitions by DMA).
  Then silu(BN2(.)), cv2, silu(BN3(.)), residual add.
"""

import threading

import numpy as np

import concourse.bass as bass
import concourse.bacc as bacc
import concourse.tile as tile
from concourse import mybir
from concourse.bass_utils import run_bass_kernel_spmd

F32 = mybir.dt.float32
BF16 = mybir.dt.bfloat16
AF = mybir.ActivationFunctionType

EPS = 1e-5
B, C1, C2, CH = 8, 256, 256, 128
H = W = 80
HW = H * W
PAD = 2
HP = H + 2 * PAD          # 84
HPP = HP * HP             # 7056
RB = 5                    # image rows per block
NPB = RB * W              # 400 pixels per block
NBLK = H // RB            # 16 blocks
NCORES = 8


def _yap(t, r, c, rows=RB):
    """View of a [128, HPP] padded-image tile: rows r..r+rows, cols c..c+W."""
    return t[:].rearrange("c (h w) -> c h w", h=HP)[:, r:r + rows, c:c + W]


def _ysrc(ypad, ypad2, r, c, rows=RB):
    """Alignment-aware source AP: even col offset -> ypad, odd -> ypad2
    (which holds ypad shifted left by one element)."""
    flat = r * HP + c
    if flat % 2 == 0:
        return _yap(ypad, r, c, rows)
    return _yap(ypad2, r, c - 1, rows)


def _rep_ap(a81, row, n):
    """Broadcast AP reading one partition row of a [81, N] tile 128 times."""
    base = a81[row:row + 1, :]
    pstep = base.ap[0][0]
    return bass.AP(base.tensor, base.offset, [[pstep, 1], [0, 128], [1, n]])


def build_program():
    nc = bacc.Bacc("TRN2", target_bir_lowering=False, debug=False)

    din = {}
    for name, shape in [
        ("x", [C1, HW]),
        ("w1T", [C1, CH]),
        ("b1", [CH, 1]),
        ("offwT", [CH, 9 * 27]),
        ("offb", [27, 1]),
        ("dyb", [27, 1]),
        ("dxb", [27, 1]),
        ("sel", [27, 81]),
        ("rep", [27, 162]),
        ("wdT", [CH, 9 * CH]),
        ("bd", [CH, 1]),
        ("w2T", [CH, C2]),
        ("b3", [CH, 2]),
    ]:
        din[name] = nc.dram_tensor(name, shape, F32, kind="ExternalInput").ap()
    dout = nc.dram_tensor("out", [C2, HW], F32, kind="ExternalOutput").ap()

    with tile.TileContext(nc) as tc:
        from contextlib import ExitStack
        with ExitStack() as ctx:
            cp = ctx.enter_context(tc.tile_pool(name="const", bufs=1))
            yp = ctx.enter_context(tc.tile_pool(name="ypadp", bufs=1))

            # ---- constants into SBUF (gpsimd DMA casts f32 -> bf16) ----
            w1a = cp.tile([128, CH], F32)
            nc.gpsimd.dma_start(w1a[:], din["w1T"][0:128, :])
            w1b = cp.tile([128, CH], F32)
            nc.gpsimd.dma_start(w1b[:], din["w1T"][128:256, :])
            b1 = cp.tile([CH, 1], F32)
            nc.gpsimd.dma_start(b1[:], din["b1"][:])
            offw = cp.tile([128, 9 * 27], BF16)
            nc.gpsimd.dma_start(offw[:], din["offwT"][:])
            offb = cp.tile([27, 1], F32)
            nc.gpsimd.dma_start(offb[:], din["offb"][:])
            dyb = cp.tile([27, 1], F32)
            nc.gpsimd.dma_start(dyb[:], din["dyb"][:])
            dxb = cp.tile([27, 1], F32)
            nc.gpsimd.dma_start(dxb[:], din["dxb"][:])
            sel = cp.tile([27, 81], BF16)
            nc.gpsimd.dma_start(sel[:], din["sel"][:])
            rep = cp.tile([27, 162], BF16)
            nc.gpsimd.dma_start(rep[:], din["rep"][:])
            wd = cp.tile([128, 9 * CH], BF16)
            nc.gpsimd.dma_start(wd[:], din["wdT"][:])
            bd = cp.tile([CH, 1], F32)
            nc.gpsimd.dma_start(bd[:], din["bd"][:])
            w2 = cp.tile([128, C2], BF16)
            nc.gpsimd.dma_start(w2[:], din["w2T"][:])
            b3 = cp.tile([CH, 2], F32)
            nc.gpsimd.dma_start(b3[:], din["b3"][:])

            ypad = yp.tile([128, HPP], BF16)
            ypad2 = yp.tile([128, HPP], BF16)
            nc.gpsimd.memset(ypad[:], 0.0)
            nc.gpsimd.memset(ypad2[:], 0.0)

            # ================= phase 1: cv1 + BN1 + silu -> ypad =========
            with tc.tile_pool(name="ph1", bufs=3) as p1, \
                 tc.tile_pool(name="ph1ps", bufs=2, space="PSUM") as pp1:
                for b in range(NBLK):
                    p0 = b * NPB
                    xa = p1.tile([128, NPB], F32, tag="xa")
                    nc.sync.dma_start(xa[:], din["x"][0:128, p0:p0 + NPB])
                    xb = p1.tile([128, NPB], F32, tag="xb")
                    nc.sync.dma_start(xb[:], din["x"][128:256, p0:p0 + NPB])
                    ps1 = pp1.tile([128, NPB], F32, tag="ps1")
                    nc.tensor.matmul(ps1[:], w1a[:], xa[:], start=True, stop=False)
                    nc.tensor.matmul(ps1[:], w1b[:], xb[:], start=False, stop=True)
                    u = p1.tile([128, NPB], F32, tag="u1")
                    nc.scalar.activation(u[:], ps1[:], AF.Identity, bias=b1[:], scale=1.0)
                    t = p1.tile([128, NPB], F32, tag="t1")
                    nc.scalar.activation(t[:], ps1[:], AF.Sigmoid, bias=b1[:], scale=1.0)
                    dstv = _yap(ypad, PAD + b * RB, PAD)
                    uv = u[:].rearrange("c (h w) -> c h w", h=RB)
                    tv = t[:].rearrange("c (h w) -> c h w", h=RB)
                    nc.vector.tensor_mul(dstv, uv, tv)

            # parity-shifted copy (ypad2[i] = ypad[i+1]) for 4B-aligned reads
            nc.vector.tensor_copy(ypad2[:, 0:HPP - 2], ypad[:, 1:HPP - 1])

            # ================= phase 2: per-block DCN + cv2 ==============
            with tc.tile_pool(name="wk", bufs=2) as wk, \
                 tc.tile_pool(name="abp", bufs=4) as abp, \
                 tc.tile_pool(name="vep", bufs=4) as vep, \
                 tc.tile_pool(name="ph2ps", bufs=1, space="PSUM") as pp2:
                for b in range(NBLK):
                    p0 = b * NPB
                    r0 = b * RB
                    # ---- offset conv (3x3, pad 1): raw[27, 400] ----
                    psR = pp2.tile([27, NPB], F32, tag="psR")
                    for n in range(9):
                        ky, kx = n // 3, n % 3
                        src = _yap(ypad, PAD + r0 + ky - 1, PAD + kx - 1)
                        nc.tensor.matmul(psR[:], offw[:, 27 * n:27 * n + 27], src,
                                         start=(n == 0), stop=(n == 8))
                    rawS = wk.tile([27, NPB], BF16, tag="rawS")
                    nc.scalar.activation(rawS[:], psR[:], AF.Identity,
                                         bias=offb[:], scale=1.0)
                    # ---- selection matmuls -> offy/offx/mask rows ----
                    psY = pp2.tile([27, NPB], F32, tag="selY")
                    nc.tensor.matmul(psY[:], sel[:, 0:27], rawS[:])
                    psX = pp2.tile([27, NPB], F32, tag="selX")
                    nc.tensor.matmul(psX[:], sel[:, 27:54], rawS[:])
                    psM = pp2.tile([27, NPB], F32, tag="selM")
                    nc.tensor.matmul(psM[:], sel[:, 54:81], rawS[:])
                    # ---- hat weights ----
                    aY = wk.tile([27, NPB], F32, tag="aY")
                    nc.scalar.activation(aY[:], psY[:], AF.Abs, bias=dyb[:], scale=1.0)
                    wY = wk.tile([27, NPB], BF16, tag="wY")
                    nc.scalar.activation(wY[:], aY[:], AF.Relu, bias=1.0, scale=-1.0)
                    aX = wk.tile([27, NPB], F32, tag="aX")
                    nc.scalar.activation(aX[:], psX[:], AF.Abs, bias=dxb[:], scale=1.0)
                    wX = wk.tile([27, NPB], BF16, tag="wX")
                    nc.scalar.activation(wX[:], aX[:], AF.Relu, bias=1.0, scale=-1.0)
                    mS = wk.tile([27, NPB], BF16, tag="mS")
                    nc.scalar.activation(mS[:], psM[:], AF.Sigmoid)
                    wYM = wk.tile([27, NPB], BF16, tag="wYM")
                    nc.vector.tensor_mul(wYM[:], wY[:], mS[:])
                    # ---- replicate to 81 rows: A = (wy*mask) x wx ----
                    psA1 = pp2.tile([81, NPB], F32, tag="selY")
                    nc.tensor.matmul(psA1[:], rep[:, 0:81], wYM[:])
                    psA2 = pp2.tile([81, NPB], F32, tag="selX")
                    nc.tensor.matmul(psA2[:], rep[:, 81:162], wX[:])
                    wx81 = wk.tile([81, NPB], BF16, tag="wx81")
                    nc.scalar.activation(wx81[:], psA2[:], AF.Copy)
                    a81 = wk.tile([81, NPB], BF16, tag="a81")
                    nc.vector.tensor_mul(a81[:], psA1[:], wx81[:])
                    # ---- 81 weighted shifted matmuls, accumulate ----
                    psO = pp2.tile([128, NPB], F32, tag="psO")
                    first = True
                    for n in range(9):
                        ky, kx = n // 3, n % 3
                        for dy in (-1, 0, 1):
                            for dx in (-1, 0, 1):
                                row = n * 9 + (dy + 1) * 3 + (dx + 1)
                                ab = abp.tile([128, NPB], BF16, tag="ab")
                                nc.sync.dma_start(ab[:], _rep_ap(a81, row, NPB))
                                ve = vep.tile([128, NPB], BF16, tag="ve")
                                src = _ysrc(ypad, ypad2,
                                            PAD + r0 + ky - 1 + dy,
                                            PAD + kx - 1 + dx)
                                nc.vector.tensor_mul(
                                    ve[:].rearrange("c (h w) -> c h w", h=RB),
                                    src,
                                    ab[:].rearrange("c (h w) -> c h w", h=RB))
                                last = (n == 8 and dy == 1 and dx == 1)
                                nc.tensor.matmul(psO[:], wd[:, CH * n:CH * (n + 1)],
                                                 ve[:], start=first, stop=last)
                                first = False
                    # ---- BN2 + silu ----
                    u2 = wk.tile([128, NPB], F32, tag="u2")
                    nc.scalar.activation(u2[:], psO[:], AF.Identity, bias=bd[:], scale=1.0)
                    t2 = wk.tile([128, NPB], F32, tag="t2")
                    nc.scalar.activation(t2[:], psO[:], AF.Sigmoid, bias=bd[:], scale=1.0)
                    s2 = wk.tile([128, NPB], BF16, tag="s2")
                    nc.vector.tensor_mul(s2[:], u2[:], t2[:])
                    # ---- cv2 + BN3 + silu + residual ----
                    for half, (tagc, ptag) in enumerate((("psC0", "o0"), ("psC1", "o1"))):
                        psC = pp2.tile([128, NPB], F32, tag=tagc)
                        nc.tensor.matmul(psC[:], w2[:, 128 * half:128 * (half + 1)], s2[:])
                        uo = wk.tile([128, NPB], F32, tag="uo" + str(half))
                        nc.scalar.activation(uo[:], psC[:], AF.Identity,
                                             bias=b3[:, half:half + 1], scale=1.0)
                        to = wk.tile([128, NPB], F32, tag="to" + str(half))
                        nc.scalar.activation(to[:], psC[:], AF.Sigmoid,
                                             bias=b3[:, half:half + 1], scale=1.0)
                        xr = wk.tile([128, NPB], F32, tag="xr" + str(half))
                        nc.sync.dma_start(
                            xr[:], din["x"][128 * half:128 * (half + 1), p0:p0 + NPB])
                        oo = wk.tile([128, NPB], F32, tag=ptag)
                        nc.vector.tensor_mul(oo[:], uo[:], to[:])
                        oo2 = wk.tile([128, NPB], F32, tag=ptag + "r")
                        nc.vector.tensor_add(oo2[:], oo[:], xr[:])
                        nc.sync.dma_start(
                            dout[128 * half:128 * (half + 1), p0:p0 + NPB], oo2[:])

    nc.compile()
    return nc


def prep_maps(inputs):
    """Host-side weight prep + per-core input maps."""
    g = lambda k: np.asarray(inputs[k], np.float32)
    x = g("x")
    inv1 = g("bn1_g") / np.sqrt(g("bn1_v") + EPS)
    w1 = g("cv1_w") * inv1[:, None]
    w1T = np.ascontiguousarray(w1.T)                       # [256, 128]
    b1 = (g("bn1_b") - g("bn1_m") * inv1).reshape(CH, 1)
    offw = g("off_w").reshape(27, CH, 9)
    offwT = np.ascontiguousarray(
        np.concatenate([offw[:, :, n].T for n in range(9)], axis=1))  # [128, 243]
    offb = g("off_b").reshape(27, 1)
    dyv = np.array([-1.0, 0.0, 1.0], np.float32)
    dyb = np.ascontiguousarray(np.tile(-dyv, 9).reshape(27, 1))
    dxb = dyb.copy()
    sel = np.zeros((27, 81), np.float32)
    for m in range(27):
        sel[2 * (m // 3), m] = 1.0            # SY
        sel[2 * (m // 3) + 1, 27 + m] = 1.0   # SX
        sel[18 + m // 3, 54 + m] = 1.0        # SM
    repm = np.zeros((27, 162), np.float32)
    for n in range(9):
        for iy in range(3):
            for ix in range(3):
                m = n * 9 + iy * 3 + ix
                repm[n * 3 + iy, m] = 1.0          # RY
                repm[n * 3 + ix, 81 + m] = 1.0     # RX
    inv2 = g("bn2_g") / np.sqrt(g("bn2_v") + EPS)
    wdf = g("dcn_w") * inv2[:, None, None, None]           # [128,128,3,3]
    wdT = np.ascontiguousarray(np.concatenate(
        [wdf[:, :, n // 3, n % 3].T for n in range(9)], axis=1))  # [128, 1152]
    bd = (g("bn2_b") - g("bn2_m") * inv2 + g("dcn_b") * inv2).reshape(CH, 1)
    inv3 = g("bn3_g") / np.sqrt(g("bn3_v") + EPS)
    w2 = g("cv2_w") * inv3[:, None]
    w2T = np.ascontiguousarray(w2.T)                       # [128, 256]
    b3f = g("bn3_b") - g("bn3_m") * inv3
    b3 = np.ascontiguousarray(np.stack([b3f[0:128], b3f[128:256]], axis=1))

    shared = dict(w1T=w1T, b1=b1, offwT=offwT, offb=offb, dyb=dyb, dxb=dxb,
                  sel=sel, rep=repm, wdT=wdT, bd=bd, w2T=w2T, b3=b3)
    maps = []
    for b in range(B):
        m = dict(shared)
        m["x"] = np.ascontiguousarray(x[b].reshape(C1, HW))
        maps.append(m)
    return maps


_CACHE = threading.local()


def get_program():
    nc = getattr(_CACHE, "nc", None)
    if nc is None:
        nc = build_program()
        _CACHE.nc = nc
    return nc


def kernel(**inputs):
    nc = get_program()
    maps = prep_maps(inputs)
    res = run_bass_kernel_spmd(nc, maps, list(range(NCORES)))
    out = np.stack([res.results[i]["out"].reshape(C2, H, W) for i in range(B)])
    return out.astype(np.float32)


if __name__ == "__main__":
    # quick shape smoke with random inputs
    rng = np.random.default_rng(0)
    fake = dict(
        x=rng.standard_normal((B, C1, H, W), dtype=np.float32),
        cv1_w=rng.standard_normal((CH, C1), dtype=np.float32) * 0.06,
        bn1_g=np.ones(CH, np.float32), bn1_b=np.zeros(CH, np.float32),
        bn1_m=np.zeros(CH, np.float32), bn1_v=np.ones(CH, np.float32),
        off_w=rng.standard_normal((27, CH, 3, 3), dtype=np.float32) * 0.01,
        off_b=np.zeros(27, np.float32),
        dcn_w=rng.standard_normal((CH, CH, 3, 3), dtype=np.float32) * 0.03,
        dcn_b=np.zeros(CH, np.float32),
        bn2_g=np.ones(CH, np.float32), bn2_b=np.zeros(CH, np.float32),
        bn2_m=np.zeros(CH, np.float32), bn2_v=np.ones(CH, np.float32),
        cv2_w=rng.standard_normal((C2, CH), dtype=np.float32) * 0.09,
        bn3_g=np.ones(C2, np.float32), bn3_b=np.zeros(C2, np.float32),
        bn3_m=np.zeros(C2, np.float32), bn3_v=np.ones(C2, np.float32),
    )
    out = kernel(**fake)
    print("kernel out:", out.shape, out.dtype, float(np.abs(out).max()))


# revision 7
# speedup vs baseline: 5.5250x; 5.5250x over previous
"""Trainium2 Bass kernel for nn_DeformBottleneck (DCNv2 bottleneck block).

Contract: kernel(**inputs) takes the FULL unsharded inputs (batch 8) and
returns the FULL [8, 256, 80, 80] output. Internally: data-parallel over
batch across 8 NeuronCores (one sample per core), SPMD via
run_bass_kernel_spmd.

Algorithm (per core / sample), all shapes hardcoded:
  y = silu(BN1(cv1 @ x))                          # [128, 6400] channel-major
  raw = off_conv3x3(y); offsets/mask from raw
  DCNv2 via exact 3x3 "hat window" reformulation of bilinear sampling:
    out[o,p] = sum_n sum_{dy,dx} A[n,dy,dx,p] * (W_n y)[o, p+(dy,dx)]
  where A = mask * relu(1-|offy-dy|) * relu(1-|offx-dx|); zero-padding
  reproduces the reference's corner-validity masking exactly.
  Implemented as 81 accumulating matmuls whose rhs is y (shifted AP)
  pre-multiplied by the per-pixel weight row (replicated across the 128
  partitions by DMA).
  Then silu(BN2(.)), cv2, silu(BN3(.)), residual add.
"""

import threading

import numpy as np

import concourse.bass as bass
import concourse.bacc as bacc
import concourse.tile as tile
from concourse import mybir
from concourse.bass_utils import run_bass_kernel_spmd

F32 = mybir.dt.float32
BF16 = mybir.dt.bfloat16
AF = mybir.ActivationFunctionType

EPS = 1e-5
B, C1, C2, CH = 8, 256, 256, 128
H = W = 80
HW = H * W
PAD = 2
HP = H + 2 * PAD          # 84
HPP = HP * HP             # 7056
RB = 5                    # image rows per weight-land sub-block
NPB = RB * W              # 400 pixels per sub-block (one PSUM bank)
RBB = 20                  # image rows per big block
NPBB = RBB * W            # 1600 pixels per big block
NSUB = RBB // RB          # 4 sub-blocks per big block
NBIG = H // RBB           # 4 big blocks
NCORES = 8


def _yap(t, r, c, rows=RB):
    """View of a [128, HPP] padded-image tile: rows r..r+rows, cols c..c+W."""
    return t[:].rearrange("c (h w) -> c h w", h=HP)[:, r:r + rows, c:c + W]


def _ysrc(ypad, ypad2, r, c, rows=RB):
    """Alignment-aware source AP: even col offset -> ypad, odd -> ypad2
    (which holds ypad shifted left by one element)."""
    flat = r * HP + c
    if flat % 2 == 0:
        return _yap(ypad, r, c, rows)
    return _yap(ypad2, r, c - 1, rows)


def _rep_ap(a81, row, n):
    """Broadcast AP reading one partition row of a [81, N] tile 128 times."""
    base = a81[row:row + 1, :]
    pstep = base.ap[0][0]
    return bass.AP(base.tensor, base.offset, [[pstep, 1], [0, 128], [1, n]])


def build_program():
    nc = bacc.Bacc("TRN2", target_bir_lowering=False, debug=False)

    din = {}
    for name, shape in [
        ("x", [C1, HW]),
        ("w1T", [C1, CH]),
        ("b1", [CH, 1]),
        ("offwT", [CH, 9 * 27]),
        ("offb", [27, 1]),
        ("dyb", [27, 1]),
        ("dxb", [27, 1]),
        ("sel", [27, 81]),
        ("rep", [27, 162]),
        ("wdT", [CH, 9 * CH]),
        ("bd", [CH, 1]),
        ("w2T", [CH, C2]),
        ("b3", [CH, 2]),
    ]:
        din[name] = nc.dram_tensor(name, shape, F32, kind="ExternalInput").ap()
    dout = nc.dram_tensor("out", [C2, HW], F32, kind="ExternalOutput").ap()

    with tile.TileContext(nc) as tc:
        from contextlib import ExitStack
        with ExitStack() as ctx:
            cp = ctx.enter_context(tc.tile_pool(name="const", bufs=1))
            yp = ctx.enter_context(tc.tile_pool(name="ypadp", bufs=1))

            # ---- constants into SBUF (gpsimd DMA casts f32 -> bf16) ----
            w1a = cp.tile([128, CH], F32)
            nc.gpsimd.dma_start(w1a[:], din["w1T"][0:128, :])
            w1b = cp.tile([128, CH], F32)
            nc.gpsimd.dma_start(w1b[:], din["w1T"][128:256, :])
            b1 = cp.tile([CH, 1], F32)
            nc.gpsimd.dma_start(b1[:], din["b1"][:])
            offw = cp.tile([128, 9 * 27], BF16)
            nc.gpsimd.dma_start(offw[:], din["offwT"][:])
            offb = cp.tile([27, 1], F32)
            nc.gpsimd.dma_start(offb[:], din["offb"][:])
            dyb = cp.tile([27, 1], F32)
            nc.gpsimd.dma_start(dyb[:], din["dyb"][:])
            dxb = cp.tile([27, 1], F32)
            nc.gpsimd.dma_start(dxb[:], din["dxb"][:])
            sel = cp.tile([27, 81], BF16)
            nc.gpsimd.dma_start(sel[:], din["sel"][:])
            rep = cp.tile([27, 162], BF16)
            nc.gpsimd.dma_start(rep[:], din["rep"][:])
            wd = cp.tile([128, 9 * CH], BF16)
            nc.gpsimd.dma_start(wd[:], din["wdT"][:])
            bd = cp.tile([CH, 1], F32)
            nc.gpsimd.dma_start(bd[:], din["bd"][:])
            w2 = cp.tile([128, C2], BF16)
            nc.gpsimd.dma_start(w2[:], din["w2T"][:])
            b3 = cp.tile([CH, 2], F32)
            nc.gpsimd.dma_start(b3[:], din["b3"][:])

            ypad = yp.tile([128, HPP], BF16)
            ypad2 = yp.tile([128, HPP], BF16)
            nc.gpsimd.memset(ypad[:], 0.0)
            nc.gpsimd.memset(ypad2[:], 0.0)

            # ================= phase 1: cv1 + BN1 + silu -> ypad =========
            with tc.tile_pool(name="ph1", bufs=3) as p1, \
                 tc.tile_pool(name="ph1ps", bufs=2, space="PSUM") as pp1:
                for b in range(H // RB):
                    p0 = b * NPB
                    xa = p1.tile([128, NPB], F32, tag="xa")
                    nc.sync.dma_start(xa[:], din["x"][0:128, p0:p0 + NPB])
                    xb = p1.tile([128, NPB], F32, tag="xb")
                    nc.sync.dma_start(xb[:], din["x"][128:256, p0:p0 + NPB])
                    ps1 = pp1.tile([128, NPB], F32, tag="ps1")
                    nc.tensor.matmul(ps1[:], w1a[:], xa[:], start=True, stop=False)
                    nc.tensor.matmul(ps1[:], w1b[:], xb[:], start=False, stop=True)
                    u = p1.tile([128, NPB], F32, tag="u1")
                    nc.scalar.activation(u[:], ps1[:], AF.Identity, bias=b1[:], scale=1.0)
                    t = p1.tile([128, NPB], F32, tag="t1")
                    nc.scalar.activation(t[:], ps1[:], AF.Sigmoid, bias=b1[:], scale=1.0)
                    dstv = _yap(ypad, PAD + b * RB, PAD)
                    uv = u[:].rearrange("c (h w) -> c h w", h=RB)
                    tv = t[:].rearrange("c (h w) -> c h w", h=RB)
                    nc.vector.tensor_mul(dstv, uv, tv)

            # parity-shifted copy (ypad2[i] = ypad[i+1]) for 4B-aligned reads
            nc.vector.tensor_copy(ypad2[:, 0:HPP - 2], ypad[:, 1:HPP - 1])

            # ================= phase 2: per-big-block DCN + cv2 ==========
            with tc.tile_pool(name="wk", bufs=2) as wk, \
                 tc.tile_pool(name="abp", bufs=4) as abp, \
                 tc.tile_pool(name="vep", bufs=4) as vep, \
                 tc.tile_pool(name="a81d", bufs=2, space="DRAM") as a81dp, \
                 tc.tile_pool(name="ph2ps", bufs=1, space="PSUM") as pp2:
                for bb in range(NBIG):
                    # ---- weight land at sub-block (400 px) granularity ----
                    a81 = wk.tile([81, NPBB], BF16, tag="a81")
                    for s in range(NSUB):
                        r0 = bb * RBB + s * RB
                        c0 = s * NPB
                        psR = pp2.tile([27, NPB], F32, tag="psR")
                        for n in range(9):
                            ky, kx = n // 3, n % 3
                            src = _yap(ypad, PAD + r0 + ky - 1, PAD + kx - 1)
                            nc.tensor.matmul(psR[:], offw[:, 27 * n:27 * n + 27], src,
                                             start=(n == 0), stop=(n == 8))
                        rawS = wk.tile([27, NPB], BF16, tag="rawS")
                        nc.scalar.activation(rawS[:], psR[:], AF.Identity,
                                             bias=offb[:], scale=1.0)
                        psY = pp2.tile([27, NPB], F32, tag="selY")
                        nc.tensor.matmul(psY[:], sel[:, 0:27], rawS[:])
                        psX = pp2.tile([27, NPB], F32, tag="selX")
                        nc.tensor.matmul(psX[:], sel[:, 27:54], rawS[:])
                        psM = pp2.tile([27, NPB], F32, tag="selM")
                        nc.tensor.matmul(psM[:], sel[:, 54:81], rawS[:])
                        aY = wk.tile([27, NPB], F32, tag="aY")
                        nc.scalar.activation(aY[:], psY[:], AF.Abs, bias=dyb[:], scale=1.0)
                        wY = wk.tile([27, NPB], BF16, tag="wY")
                        nc.scalar.activation(wY[:], aY[:], AF.Relu, bias=1.0, scale=-1.0)
                        aX = wk.tile([27, NPB], F32, tag="aX")
                        nc.scalar.activation(aX[:], psX[:], AF.Abs, bias=dxb[:], scale=1.0)
                        wX = wk.tile([27, NPB], BF16, tag="wX")
                        nc.scalar.activation(wX[:], aX[:], AF.Relu, bias=1.0, scale=-1.0)
                        mS = wk.tile([27, NPB], BF16, tag="mS")
                        nc.scalar.activation(mS[:], psM[:], AF.Sigmoid)
                        wYM = wk.tile([27, NPB], BF16, tag="wYM")
                        nc.vector.tensor_mul(wYM[:], wY[:], mS[:])
                        psA1 = pp2.tile([81, NPB], F32, tag="selY")
                        nc.tensor.matmul(psA1[:], rep[:, 0:81], wYM[:])
                        psA2 = pp2.tile([81, NPB], F32, tag="selX")
                        nc.tensor.matmul(psA2[:], rep[:, 81:162], wX[:])
                        wx81 = wk.tile([81, NPB], BF16, tag="wx81")
                        nc.scalar.activation(wx81[:], psA2[:], AF.Copy)
                        nc.vector.tensor_mul(a81[:, c0:c0 + NPB], psA1[:], wx81[:])
                    # bounce A81 through DRAM so replication reads hit HBM
                    # (same-row SBUF reads serialize on one partition's port)
                    a81d = a81dp.tile([81, NPBB], BF16, tag="a81d")
                    nc.sync.dma_start(a81d[:], a81[:])
                    # ---- 81 weighted shifted matmuls, accumulate ----
                    psO = [pp2.tile([128, NPB], F32, tag=f"psO{k}", name=f"psO{k}")
                           for k in range(NSUB)]
                    first = True
                    for n in range(9):
                        ky, kx = n // 3, n % 3
                        for dy in (-1, 0, 1):
                            for dx in (-1, 0, 1):
                                row = n * 9 + (dy + 1) * 3 + (dx + 1)
                                ab = abp.tile([128, NPBB], BF16, tag="ab")
                                eng = nc.sync if (row % 2 == 0) else nc.scalar
                                eng.dma_start(ab[:], _rep_ap(a81d, row, NPBB))
                                ve = vep.tile([128, NPBB], BF16, tag="ve")
                                src = _ysrc(ypad, ypad2,
                                            PAD + bb * RBB + ky - 1 + dy,
                                            PAD + kx - 1 + dx, rows=RBB)
                                nc.vector.tensor_mul(
                                    ve[:].rearrange("c (h w) -> c h w", h=RBB),
                                    src,
                                    ab[:].rearrange("c (h w) -> c h w", h=RBB))
                                last = (n == 8 and dy == 1 and dx == 1)
                                for k in range(NSUB):
                                    nc.tensor.matmul(
                                        psO[k][:], wd[:, CH * n:CH * (n + 1)],
                                        ve[:, k * NPB:(k + 1) * NPB],
                                        start=first, stop=last)
                                first = False
                    # ---- BN2 + silu ; cv2 + BN3 + silu + residual per sub ----
                    for k in range(NSUB):
                        p0 = bb * NPBB + k * NPB
                        u2 = wk.tile([128, NPB], F32, tag="u2")
                        nc.scalar.activation(u2[:], psO[k][:], AF.Identity,
                                             bias=bd[:], scale=1.0)
                        t2 = wk.tile([128, NPB], F32, tag="t2")
                        nc.scalar.activation(t2[:], psO[k][:], AF.Sigmoid,
                                             bias=bd[:], scale=1.0)
                        s2 = wk.tile([128, NPB], BF16, tag="s2")
                        nc.vector.tensor_mul(s2[:], u2[:], t2[:])
                        for half, tagc in enumerate(("selY", "selX")):
                            psC = pp2.tile([128, NPB], F32, tag=tagc)
                            nc.tensor.matmul(psC[:],
                                             w2[:, 128 * half:128 * (half + 1)], s2[:])
                            uo = wk.tile([128, NPB], F32, tag="uo" + str(half))
                            nc.scalar.activation(uo[:], psC[:], AF.Identity,
                                                 bias=b3[:, half:half + 1], scale=1.0)
                            to = wk.tile([128, NPB], F32, tag="to" + str(half))
                            nc.scalar.activation(to[:], psC[:], AF.Sigmoid,
                                                 bias=b3[:, half:half + 1], scale=1.0)
                            xr = wk.tile([128, NPB], F32, tag="xr" + str(half))
                            nc.sync.dma_start(
                                xr[:], din["x"][128 * half:128 * (half + 1),
                                                p0:p0 + NPB])
                            oo = wk.tile([128, NPB], F32, tag="oo" + str(half))
                            nc.vector.tensor_mul(oo[:], uo[:], to[:])
                            oo2 = wk.tile([128, NPB], F32, tag="or" + str(half))
                            nc.vector.tensor_add(oo2[:], oo[:], xr[:])
                            nc.sync.dma_start(
                                dout[128 * half:128 * (half + 1), p0:p0 + NPB],
                                oo2[:])

    nc.compile()
    return nc


def prep_maps(inputs):
    """Host-side weight prep + per-core input maps."""
    g = lambda k: np.asarray(inputs[k], np.float32)
    x = g("x")
    inv1 = g("bn1_g") / np.sqrt(g("bn1_v") + EPS)
    w1 = g("cv1_w") * inv1[:, None]
    w1T = np.ascontiguousarray(w1.T)                       # [256, 128]
    b1 = (g("bn1_b") - g("bn1_m") * inv1).reshape(CH, 1)
    offw = g("off_w").reshape(27, CH, 9)
    offwT = np.ascontiguousarray(
        np.concatenate([offw[:, :, n].T for n in range(9)], axis=1))  # [128, 243]
    offb = g("off_b").reshape(27, 1)
    dyv = np.array([-1.0, 0.0, 1.0], np.float32)
    dyb = np.ascontiguousarray(np.tile(-dyv, 9).reshape(27, 1))
    dxb = dyb.copy()
    sel = np.zeros((27, 81), np.float32)
    for m in range(27):
        sel[2 * (m // 3), m] = 1.0            # SY
        sel[2 * (m // 3) + 1, 27 + m] = 1.0   # SX
        sel[18 + m // 3, 54 + m] = 1.0        # SM
    repm = np.zeros((27, 162), np.float32)
    for n in range(9):
        for iy in range(3):
            for ix in range(3):
                m = n * 9 + iy * 3 + ix
                repm[n * 3 + iy, m] = 1.0          # RY
                repm[n * 3 + ix, 81 + m] = 1.0     # RX
    inv2 = g("bn2_g") / np.sqrt(g("bn2_v") + EPS)
    wdf = g("dcn_w") * inv2[:, None, None, None]           # [128,128,3,3]
    wdT = np.ascontiguousarray(np.concatenate(
        [wdf[:, :, n // 3, n % 3].T for n in range(9)], axis=1))  # [128, 1152]
    bd = (g("bn2_b") - g("bn2_m") * inv2 + g("dcn_b") * inv2).reshape(CH, 1)
    inv3 = g("bn3_g") / np.sqrt(g("bn3_v") + EPS)
    w2 = g("cv2_w") * inv3[:, None]
    w2T = np.ascontiguousarray(w2.T)                       # [128, 256]
    b3f = g("bn3_b") - g("bn3_m") * inv3
    b3 = np.ascontiguousarray(np.stack([b3f[0:128], b3f[128:256]], axis=1))

    shared = dict(w1T=w1T, b1=b1, offwT=offwT, offb=offb, dyb=dyb, dxb=dxb,
                  sel=sel, rep=repm, wdT=wdT, bd=bd, w2T=w2T, b3=b3)
    maps = []
    for b in range(B):
        m = dict(shared)
        m["x"] = np.ascontiguousarray(x[b].reshape(C1, HW))
        maps.append(m)
    return maps


_CACHE = threading.local()


def get_program():
    nc = getattr(_CACHE, "nc", None)
    if nc is None:
        nc = build_program()
        _CACHE.nc = nc
    return nc


def kernel(**inputs):
    nc = get_program()
    maps = prep_maps(inputs)
    res = run_bass_kernel_spmd(nc, maps, list(range(NCORES)))
    out = np.stack([res.results[i]["out"].reshape(C2, H, W) for i in range(B)])
    return out.astype(np.float32)


if __name__ == "__main__":
    # quick shape smoke with random inputs
    rng = np.random.default_rng(0)
    fake = dict(
        x=rng.standard_normal((B, C1, H, W), dtype=np.float32),
        cv1_w=rng.standard_normal((CH, C1), dtype=np.float32) * 0.06,
        bn1_g=np.ones(CH, np.float32), bn1_b=np.zeros(CH, np.float32),
        bn1_m=np.zeros(CH, np.float32), bn1_v=np.ones(CH, np.float32),
        off_w=rng.standard_normal((27, CH, 3, 3), dtype=np.float32) * 0.01,
        off_b=np.zeros(27, np.float32),
        dcn_w=rng.standard_normal((CH, CH, 3, 3), dtype=np.float32) * 0.03,
        dcn_b=np.zeros(CH, np.float32),
        bn2_g=np.ones(CH, np.float32), bn2_b=np.zeros(CH, np.float32),
        bn2_m=np.zeros(CH, np.float32), bn2_v=np.ones(CH, np.float32),
        cv2_w=rng.standard_normal((C2, CH), dtype=np.float32) * 0.09,
        bn3_g=np.ones(C2, np.float32), bn3_b=np.zeros(C2, np.float32),
        bn3_m=np.zeros(C2, np.float32), bn3_v=np.ones(C2, np.float32),
    )
    out = kernel(**fake)
    print("kernel out:", out.shape, out.dtype, float(np.abs(out).max()))
